# revision 2
# baseline (speedup 1.0000x reference)
"""Multi-head attention (b=4, n=2048, dim=768, 12 heads) on 8 TRN2 NeuronCores.

Sharding: core c handles batch c//2 and head-group c%2 (6 of 12 heads).  Each
core computes its heads' contribution projected through its slice of Wo and
returns a partial [2048, 768] f32 output; the host sums core pairs and adds
the bias.  No on-device collectives needed.

v2: fused pipeline.  The attention inner loop (P2: scores -> exp -> A@V) is
ACT(exp)-and-PE co-limited; Q-projection and the output stage (P3) are
interleaved into P2's PE slack instead of running as separate phases:
  - units ordered ib-outer so OPS[:, ib] completes at the end of window ib;
    P3(ib) chunks and Q-proj(ib+1) chunks are emitted between P2 chunks of
    window ib+1 and the Tile scheduler slides them into PE gaps.
  - PSUM (8 banks): stA [128,1536] (3) + stB [128,1024] (2) alternating for
    score tiles, plus one shared 3-slot [128,512] pool for AV accumulation,
    Q-proj accumulation, and all P3 psum tiles.
  - out-proj PSUM->SBUF copies moved from ACT to DVE (ACT runs only exp).
Head computes K,V (all tokens) and Q(ib0); tail runs P3(ib3).
"""
import os
import sys
import types
import numpy as np
import ml_dtypes

B, N, DIM = 4, 2048, 768
HEADS, DH = 12, 64
HPC = 6                # heads per core
FPC = HPC * DH         # 384 features per core
NCORES = 8
KC = DIM // 128        # 6 contraction chunks
FT = FPC // 128        # 3 feature tiles per core
NT = N // 128          # 16 token chunks of 128
IBS = 512              # i-block size
IB = N // IBS          # 4 i-blocks
BF16 = ml_dtypes.bfloat16

_cache = {}
last_exec_time_ns = None


def _install_ntff_hook():
    try:
        import antenv.axon_hooks  # noqa: F401
        return
    except ImportError:
        pass
    from trn_agent_boot.trn_boot import _ntff_profile_via_ctypes
    hook = _ntff_profile_via_ctypes('/opt/axon/libaxon_pjrt.so')
    mod = types.ModuleType('antenv.axon_hooks')
    mod.get_axon_ntff_profile_hook = lambda: hook
    import antenv
    sys.modules['antenv.axon_hooks'] = mod
    antenv.axon_hooks = mod


def _build_nc():
    from contextlib import ExitStack
    from concourse import bacc
    import concourse.mybir as mybir
    from concourse.tile import TileContext
    from concourse.masks import make_identity
    from concourse.bass import broadcast_tensor_aps

    dt = mybir.dt
    EXP = mybir.ActivationFunctionType.Exp

    nc = bacc.Bacc("TRN2", target_bir_lowering=False, debug=False,
                   num_devices=NCORES)
    xT = nc.dram_tensor("xT", [DIM, N], dt.bfloat16, kind="ExternalInput").ap()
    wq = nc.dram_tensor("wq", [DIM, FPC], dt.bfloat16, kind="ExternalInput").ap()
    wk = nc.dram_tensor("wk", [DIM, FPC], dt.bfloat16, kind="ExternalInput").ap()
    wv = nc.dram_tensor("wv", [DIM, FPC], dt.bfloat16, kind="ExternalInput").ap()
    wo = nc.dram_tensor("wo", [FPC, DIM], dt.bfloat16, kind="ExternalInput").ap()
    out = nc.dram_tensor("out", [N, DIM], dt.float32, kind="ExternalOutput").ap()

    with TileContext(nc) as tc, ExitStack() as ctx:
        const = ctx.enter_context(tc.tile_pool(name="const", bufs=1))
        id_f32 = const.tile([128, 128], dt.float32, tag="idf")
        id_bf = const.tile([128, 128], dt.bfloat16, tag="idb")
        make_identity(nc, id_f32)
        make_identity(nc, id_bf)

        inp = ctx.enter_context(tc.tile_pool(name="inp", bufs=1))
        xts2 = [[inp.tile([128, N // 2], dt.bfloat16, tag=f"xt{k}_{hf}",
                          name=f"xt{k}_{hf}") for hf in range(2)]
                for k in range(KC)]
        wqs = [inp.tile([128, FPC], dt.bfloat16, tag=f"wq{k}", name=f"wq{k}")
               for k in range(KC)]
        wks = [inp.tile([128, FPC], dt.bfloat16, tag=f"wk{k}", name=f"wk{k}")
               for k in range(KC)]
        wvs = [inp.tile([128, FPC], dt.bfloat16, tag=f"wv{k}", name=f"wv{k}")
               for k in range(KC)]
        wos = [inp.tile([128, DIM], dt.bfloat16, tag=f"wo{f}", name=f"wo{f}")
               for f in range(FT)]
        for k in range(KC):
            nc.sync.dma_start(out=xts2[k][0][:],
                              in_=xT[k * 128:(k + 1) * 128, 0:N // 2])
            nc.scalar.dma_start(out=wvs[k][:], in_=wv[k * 128:(k + 1) * 128, :])
        for k in range(KC):
            nc.sync.dma_start(out=xts2[k][1][:],
                              in_=xT[k * 128:(k + 1) * 128, N // 2:N])
        for k in range(KC):
            nc.sync.dma_start(out=wks[k][:], in_=wk[k * 128:(k + 1) * 128, :])
            nc.sync.dma_start(out=wqs[k][:], in_=wq[k * 128:(k + 1) * 128, :])
        for f in range(FT):
            nc.scalar.dma_start(out=wos[f][:], in_=wo[f * 128:(f + 1) * 128, :])

        kqv = ctx.enter_context(tc.tile_pool(name="kqv", bufs=1))
        KT = [kqv.tile([128, N], dt.bfloat16, tag=f"kt{f}", name=f"kt{f}")
              for f in range(FT)]
        QT = [kqv.tile([128, N], dt.bfloat16, tag=f"qt{f}", name=f"qt{f}")
              for f in range(FT)]
        VP = [kqv.tile([128, HPC * 128], dt.bfloat16, tag=f"vp{t}", name=f"vp{t}")
              for t in range(NT)]
        opsb = ctx.enter_context(tc.tile_pool(name="opsb", bufs=1))
        OPS = [[opsb.tile([65, IBS], dt.float32, tag=f"op{h}_{ib}",
                          name=f"op{h}_{ib}") for ib in range(IB)]
               for h in range(HPC)]

        # persistent SBUF pools for P3 / Q-proj filler
        otokp = ctx.enter_context(tc.tile_pool(name="otokp", bufs=3))
        otnp = ctx.enter_context(tc.tile_pool(name="otnp", bufs=4))
        linvp = ctx.enter_context(tc.tile_pool(name="linvp", bufs=3))
        outst = ctx.enter_context(tc.tile_pool(name="outst", bufs=3))
        expp = ctx.enter_context(tc.tile_pool(name="expp", bufs=4))

        # ---- head: V, K, Q(ib0) projections ----
        for t in range(NT):
            nc.vector.memset(
                VP[t].rearrange("p (h c) -> p h c", c=128)[:, :, 64:65], 1.0)
        with tc.tile_pool(name="p1ps", bufs=3, space="PSUM") as p1:
            for t in range(NT):
                ps = p1.tile([128, FPC], dt.float32, tag="p1", name=f"vps{t}")
                for k in range(KC):
                    nc.tensor.matmul(
                        ps[:],
                        lhsT=xts2[k][t // 8][:, (t % 8) * 128:(t % 8 + 1) * 128],
                        rhs=wvs[k][:], start=(k == 0), stop=(k == KC - 1))
                nc.vector.tensor_copy(
                    VP[t].rearrange("p (h c) -> p h c", c=128)[:, :, 0:64],
                    ps.rearrange("p (h c) -> p h c", c=64))
            for f in range(FT):
                for q in range(N // 512):
                    ps = p1.tile([128, 512], dt.float32, tag="p1",
                                 name=f"kps{f}_{q}")
                    for k in range(KC):
                        nc.tensor.matmul(
                            ps[:], lhsT=wks[k][:, f * 128:(f + 1) * 128],
                            rhs=xts2[k][q // 2][:, (q % 2) * 512:
                                                (q % 2 + 1) * 512],
                            start=(k == 0), stop=(k == KC - 1))
                    nc.vector.tensor_copy(KT[f][:, q * 512:(q + 1) * 512],
                                          ps[:])
            for f in range(FT):
                ps = p1.tile([128, 512], dt.float32, tag="p1", name=f"qps{f}_0")
                for k in range(KC):
                    nc.tensor.matmul(
                        ps[:], lhsT=wqs[k][:, f * 128:(f + 1) * 128],
                        rhs=xts2[k][0][:, 0:512],
                        start=(k == 0), stop=(k == KC - 1))
                nc.vector.tensor_copy(QT[f][:, 0:512], ps[:])

        # ---- fused main loop: P2 + interleaved Q-proj / P3 filler ----
        stA = ctx.enter_context(tc.tile_pool(name="stA", bufs=1, space="PSUM"))
        stB = ctx.enter_context(tc.tile_pool(name="stB", bufs=1, space="PSUM"))
        acc = ctx.enter_context(tc.tile_pool(name="acc", bufs=3, space="PSUM"))

        otoks = {}
        otns = {}
        obs = {}

        def qp_chunk(f, q):
            ps = acc.tile([128, 512], dt.float32, tag="acc", name=f"qps{f}_{q}")
            for k in range(KC):
                nc.tensor.matmul(
                    ps[:], lhsT=wqs[k][:, f * 128:(f + 1) * 128],
                    rhs=xts2[k][q // 2][:, (q % 2) * 512:(q % 2 + 1) * 512],
                    start=(k == 0), stop=(k == KC - 1))
            nc.vector.tensor_copy(QT[f][:, q * 512:(q + 1) * 512], ps[:])

        def tr_chunk(isub):
            ib, col = isub // 4, (isub % 4) * 128
            trp = acc.tile([128, HPC * 65], dt.float32, tag="acc",
                           name=f"trp{isub}")
            for h in range(HPC):
                nc.tensor.transpose(trp[:, h * 65:(h + 1) * 65],
                                    OPS[h][ib][0:65, col:col + 128],
                                    id_f32[0:65, 0:65])
            trv = trp.rearrange("p (h c) -> p h c", c=65)
            linv6 = linvp.tile([128, HPC, 1], dt.float32, tag="l6",
                               name=f"l6{isub}")
            nc.vector.reciprocal(linv6[:], trv[:, :, 64:65])
            otok = otokp.tile([128, FPC], dt.bfloat16, tag="otok",
                              name=f"otok{isub}")
            a, b = broadcast_tensor_aps(trv[:, :, 0:64], linv6[:])
            nc.vector.tensor_mul(
                otok.rearrange("p (h c) -> p h c", c=64), a, b)
            otoks[isub] = otok

        def tb_chunk(isub):
            otok = otoks.pop(isub)
            tbp = acc.tile([128, FPC], dt.bfloat16, tag="acc",
                           name=f"tbp{isub}")
            for f in range(FT):
                nc.tensor.transpose(tbp[:, f * 128:(f + 1) * 128],
                                    otok[:, f * 128:(f + 1) * 128],
                                    id_bf[:])
            otn = otnp.tile([128, FPC], dt.bfloat16, tag="otn",
                            name=f"otn{isub}")
            nc.vector.tensor_copy(otn[:], tbp[:])
            otns[isub] = otn

        def pj_chunk(isub, half):
            otn = otns[isub]
            if half == 0:
                obs[isub] = outst.tile([128, DIM], dt.float32, tag="ob",
                                       name=f"ob{isub}")
            ob = obs[isub]
            pp = acc.tile([128, DIM // 2], dt.float32, tag="acc",
                          name=f"pp{isub}_{half}")
            for f in range(FT):
                nc.tensor.matmul(
                    pp[:], lhsT=otn[:, f * 128:(f + 1) * 128],
                    rhs=wos[f][:, half * 384:(half + 1) * 384],
                    start=(f == 0), stop=(f == FT - 1))
            nc.vector.tensor_copy(ob[:, half * 384:(half + 1) * 384], pp[:])
            if half == 1:
                otns.pop(isub)
                nc.sync.dma_start(out=out[isub * 128:(isub + 1) * 128, :],
                                  in_=obs.pop(isub)[:])

        def window_fillers(w):
            fills = []
            qps = [(f, w + 1) for f in range(FT)] if w + 1 < IB else []
            p3 = []
            if w >= 1:
                for i in range(4):
                    isub = 4 * (w - 1) + i
                    p3.append((tr_chunk, (isub,)))
                    p3.append((tb_chunk, (isub,)))
                    p3.append((pj_chunk, (isub, 0)))
                    p3.append((pj_chunk, (isub, 1)))
            # interleave qp early among p3 chunks
            for j, qpq in enumerate(qps):
                p3.insert(min(len(p3), 1 + 5 * j), (qp_chunk, qpq))
            return p3

        # units: ib-outer so OPS[:, ib] completes at end of window ib
        units = [(h, ib, jc) for ib in range(IB) for h in range(HPC)
                 for jc in range(NT)]
        chunks = []
        i = 0
        tog = True
        while i < len(units):
            n = min(3 if tog else 2, len(units) - i)
            chunks.append(units[i:i + n])
            i += n
            tog = not tog

        ops = {}
        fillers = window_fillers(0)
        cur_w = 0
        since_fill = 0
        for g, pack in enumerate(chunks):
            w = units[sum(len(c) for c in chunks[:g])][1] if False else pack[0][1]
            if w != cur_w:
                # flush remaining fillers of previous window, load new list
                for fn, args in fillers:
                    fn(*args)
                fillers = window_fillers(w)
                cur_w = w
                since_fill = 0
            n = len(pack)
            pool = stA if n == 3 else stB
            st = pool.tile([128, n * IBS], dt.float32,
                           tag="stA" if n == 3 else "stB", name=f"st{g}")
            for u, (h, ib, jc) in enumerate(pack):
                ktf, qtf, r0 = KT[h // 2], QT[h // 2], (h % 2) * 64
                nc.tensor.matmul(
                    st[:, u * IBS:(u + 1) * IBS],
                    lhsT=ktf[r0:r0 + 64, jc * 128:(jc + 1) * 128],
                    rhs=qtf[r0:r0 + 64, ib * IBS:(ib + 1) * IBS],
                    start=True, stop=True)
            ex = expp.tile([128, n * IBS], dt.bfloat16, tag="ex",
                           name=f"ex{g}")
            nc.scalar.activation(ex[:], st[:], EXP, scale=0.125)
            for u, (h, ib, jc) in enumerate(pack):
                if jc == 0:
                    ops[(h, ib)] = acc.tile([128, IBS], dt.float32,
                                            tag="acc", name=f"opp{h}_{ib}")
                nc.tensor.matmul(
                    ops[(h, ib)][:], lhsT=VP[jc][:, h * 128:(h + 1) * 128],
                    rhs=ex[:, u * IBS:(u + 1) * IBS],
                    start=(jc == 0), stop=(jc == NT - 1))
                if jc == NT - 1:
                    op = ops.pop((h, ib))
                    nc.vector.tensor_copy(OPS[h][ib][:], op[0:65, :])
            since_fill += 1
            if since_fill >= 2 and fillers:
                fn, args = fillers.pop(0)
                fn(*args)
                since_fill = 0
        for fn, args in fillers:
            fn(*args)
        # tail: P3 for the last window
        for i in range(4):
            isub = 4 * (IB - 1) + i
            tr_chunk(isub)
            tb_chunk(isub)
            pj_chunk(isub, 0)
            pj_chunk(isub, 1)

    nc.finalize()
    return nc


def _get_nc():
    if "nc" not in _cache:
        _cache["nc"] = _build_nc()
    return _cache["nc"]


def kernel(x, Wq, Wk, Wv, Wo, bo):
    global last_exec_time_ns
    x = np.asarray(x, dtype=np.float32)
    Wq = np.asarray(Wq, dtype=np.float32)
    Wk = np.asarray(Wk, dtype=np.float32)
    Wv = np.asarray(Wv, dtype=np.float32)
    Wo = np.asarray(Wo, dtype=np.float32)
    bo = np.asarray(bo, dtype=np.float32)

    trace = bool(os.environ.get("BASS_KERNEL_TRACE"))
    if trace:
        _install_ntff_hook()
        import concourse.bass_utils as bass_utils
        bass_utils.upload_artifacts = lambda tmpdir: tmpdir

    nc = _get_nc()
    in_maps = []
    for c in range(NCORES):
        bi, hg = divmod(c, 2)
        s = slice(hg * FPC, (hg + 1) * FPC)
        in_maps.append({
            "xT": np.ascontiguousarray(x[bi].T).astype(BF16),
            "wq": np.ascontiguousarray(Wq[:, s]).astype(BF16),
            "wk": np.ascontiguousarray(Wk[:, s]).astype(BF16),
            "wv": np.ascontiguousarray(Wv[:, s]).astype(BF16),
            "wo": np.ascontiguousarray(Wo[s, :]).astype(BF16),
        })

    from concourse.bass_utils import run_bass_kernel_spmd
    res = run_bass_kernel_spmd(nc, in_maps, list(range(NCORES)), trace=trace)
    last_exec_time_ns = res.exec_time_ns

    parts = [res.results[c]["out"] for c in range(NCORES)]
    full = np.empty((B, N, DIM), np.float32)
    for bi in range(B):
        full[bi] = parts[2 * bi] + parts[2 * bi + 1] + bo[None, :]
    return full


# revision 6
# speedup vs baseline: 1.2146x; 1.2146x over previous
"""Multi-head attention (b=4, n=2048, dim=768, 12 heads) on 8 TRN2 NeuronCores.

Sharding: core c handles batch c//2 and head-group c%2 (6 of 12 heads).  Each
core computes its heads' contribution projected through its slice of Wo and
returns a partial [2048, 768] f32 output; the host sums core pairs and adds
the bias.  No on-device collectives needed.

v2: fused pipeline.  The attention inner loop (P2: scores -> exp -> A@V) is
ACT(exp)-and-PE co-limited; Q-projection and the output stage (P3) are
interleaved into P2's PE slack instead of running as separate phases:
  - units ordered ib-outer so OPS[:, ib] completes at the end of window ib;
    P3(ib) chunks and Q-proj(ib+1) chunks are emitted between P2 chunks of
    window ib+1 and the Tile scheduler slides them into PE gaps.
  - PSUM (8 banks): stA [128,1536] (3) + stB [128,1024] (2) alternating for
    score tiles, plus one shared 3-slot [128,512] pool for AV accumulation,
    Q-proj accumulation, and all P3 psum tiles.
  - out-proj PSUM->SBUF copies moved from ACT to DVE (ACT runs only exp).
Head computes K,V (all tokens) and Q(ib0); tail runs P3(ib3).
"""
import os
import sys
import types
import numpy as np
import ml_dtypes

B, N, DIM = 4, 2048, 768
HEADS, DH = 12, 64
HPC = 6                # heads per core
FPC = HPC * DH         # 384 features per core
NCORES = 8
KC = DIM // 128        # 6 contraction chunks
FT = FPC // 128        # 3 feature tiles per core
NT = N // 128          # 16 token chunks of 128
IBS = 512              # i-block size
IB = N // IBS          # 4 i-blocks
BF16 = ml_dtypes.bfloat16

_cache = {}
last_exec_time_ns = None


def _install_ntff_hook():
    try:
        import antenv.axon_hooks  # noqa: F401
        return
    except ImportError:
        pass
    from trn_agent_boot.trn_boot import _ntff_profile_via_ctypes
    hook = _ntff_profile_via_ctypes('/opt/axon/libaxon_pjrt.so')
    mod = types.ModuleType('antenv.axon_hooks')
    mod.get_axon_ntff_profile_hook = lambda: hook
    import antenv
    sys.modules['antenv.axon_hooks'] = mod
    antenv.axon_hooks = mod


def _build_nc():
    from contextlib import ExitStack
    from concourse import bacc
    import concourse.mybir as mybir
    from concourse.tile import TileContext
    from concourse.masks import make_identity
    from concourse.bass import broadcast_tensor_aps

    dt = mybir.dt
    EXP = mybir.ActivationFunctionType.Exp

    nc = bacc.Bacc("TRN2", target_bir_lowering=False, debug=False,
                   num_devices=NCORES)
    xT = nc.dram_tensor("xT", [DIM, N], dt.bfloat16, kind="ExternalInput").ap()
    wq = nc.dram_tensor("wq", [DIM, FPC], dt.bfloat16, kind="ExternalInput").ap()
    wk = nc.dram_tensor("wk", [DIM, FPC], dt.bfloat16, kind="ExternalInput").ap()
    wv = nc.dram_tensor("wv", [DIM, FPC], dt.bfloat16, kind="ExternalInput").ap()
    wo = nc.dram_tensor("wo", [FPC, DIM], dt.bfloat16, kind="ExternalInput").ap()
    out = nc.dram_tensor("out", [N, DIM], dt.float32, kind="ExternalOutput").ap()

    with TileContext(nc) as tc, ExitStack() as ctx:
        const = ctx.enter_context(tc.tile_pool(name="const", bufs=1))
        id_f32 = const.tile([128, 128], dt.float32, tag="idf")
        id_bf = const.tile([128, 128], dt.bfloat16, tag="idb")
        make_identity(nc, id_f32)
        make_identity(nc, id_bf)

        inp = ctx.enter_context(tc.tile_pool(name="inp", bufs=1))
        xts2 = [[inp.tile([128, N // 2], dt.bfloat16, tag=f"xt{k}_{hf}",
                          name=f"xt{k}_{hf}") for hf in range(2)]
                for k in range(KC)]
        wqs = [inp.tile([128, FPC], dt.bfloat16, tag=f"wq{k}", name=f"wq{k}")
               for k in range(KC)]
        wks = [inp.tile([128, FPC], dt.bfloat16, tag=f"wk{k}", name=f"wk{k}")
               for k in range(KC)]
        wvs = [inp.tile([128, FPC], dt.bfloat16, tag=f"wv{k}", name=f"wv{k}")
               for k in range(KC)]
        wos = [inp.tile([128, DIM], dt.bfloat16, tag=f"wo{f}", name=f"wo{f}")
               for f in range(FT)]
        for k in range(KC):
            nc.sync.dma_start(out=xts2[k][0][:],
                              in_=xT[k * 128:(k + 1) * 128, 0:N // 2])
            nc.scalar.dma_start(out=wvs[k][:], in_=wv[k * 128:(k + 1) * 128, :])
        for k in range(KC):
            nc.sync.dma_start(out=xts2[k][1][:],
                              in_=xT[k * 128:(k + 1) * 128, N // 2:N])
        for k in range(KC):
            nc.sync.dma_start(out=wks[k][:], in_=wk[k * 128:(k + 1) * 128, :])
            nc.sync.dma_start(out=wqs[k][:], in_=wq[k * 128:(k + 1) * 128, :])
        for f in range(FT):
            nc.scalar.dma_start(out=wos[f][:], in_=wo[f * 128:(f + 1) * 128, :])

        kqv = ctx.enter_context(tc.tile_pool(name="kqv", bufs=1))
        KT = [kqv.tile([128, N], dt.bfloat16, tag=f"kt{f}", name=f"kt{f}")
              for f in range(FT)]
        # Q tiles split per (f, ib) so the in-window Q-proj filler writes
        # never alias tiles the current window's score matmuls are reading.
        QT = [[kqv.tile([128, IBS], dt.bfloat16, tag=f"qt{f}_{q}",
                        name=f"qt{f}_{q}") for q in range(IB)]
              for f in range(FT)]
        VP = [kqv.tile([128, HPC * 128], dt.bfloat16, tag=f"vp{t}", name=f"vp{t}")
              for t in range(NT)]
        opsb = ctx.enter_context(tc.tile_pool(name="opsb", bufs=1))
        OPS = [[opsb.tile([65, IBS], dt.float32, tag=f"op{h}_{ib}",
                          name=f"op{h}_{ib}") for ib in range(IB)]
               for h in range(HPC)]

        # persistent SBUF pools for P3 / Q-proj filler
        otokp = ctx.enter_context(tc.tile_pool(name="otokp", bufs=3))
        otnp = ctx.enter_context(tc.tile_pool(name="otnp", bufs=4))
        linvp = ctx.enter_context(tc.tile_pool(name="linvp", bufs=3))
        outst = ctx.enter_context(tc.tile_pool(name="outst", bufs=3))
        expp = ctx.enter_context(tc.tile_pool(name="expp", bufs=4))

        # ---- head: V, K, Q(ib0) projections ----
        for t in range(NT):
            nc.vector.memset(
                VP[t].rearrange("p (h c) -> p h c", c=128)[:, :, 64:65], 1.0)
        with tc.tile_pool(name="p1ps", bufs=3, space="PSUM") as p1:
            for t in range(NT):
                ps = p1.tile([128, FPC], dt.float32, tag="p1", name=f"vps{t}")
                for k in range(KC):
                    nc.tensor.matmul(
                        ps[:],
                        lhsT=xts2[k][t // 8][:, (t % 8) * 128:(t % 8 + 1) * 128],
                        rhs=wvs[k][:], start=(k == 0), stop=(k == KC - 1))
                nc.vector.tensor_copy(
                    VP[t].rearrange("p (h c) -> p h c", c=128)[:, :, 0:64],
                    ps.rearrange("p (h c) -> p h c", c=64))
            for f in range(FT):
                for q in range(N // 512):
                    ps = p1.tile([128, 512], dt.float32, tag="p1",
                                 name=f"kps{f}_{q}")
                    for k in range(KC):
                        nc.tensor.matmul(
                            ps[:], lhsT=wks[k][:, f * 128:(f + 1) * 128],
                            rhs=xts2[k][q // 2][:, (q % 2) * 512:
                                                (q % 2 + 1) * 512],
                            start=(k == 0), stop=(k == KC - 1))
                    nc.vector.tensor_copy(KT[f][:, q * 512:(q + 1) * 512],
                                          ps[:])
            for f in range(FT):
                ps = p1.tile([128, 512], dt.float32, tag="p1", name=f"qps{f}_0")
                for k in range(KC):
                    nc.tensor.matmul(
                        ps[:], lhsT=wqs[k][:, f * 128:(f + 1) * 128],
                        rhs=xts2[k][0][:, 0:512],
                        start=(k == 0), stop=(k == KC - 1))
                nc.vector.tensor_copy(QT[f][0][:], ps[:])

        # ---- fused main loop: P2 + interleaved Q-proj / P3 filler ----
        stA = ctx.enter_context(tc.tile_pool(name="stA", bufs=1, space="PSUM"))
        stB = ctx.enter_context(tc.tile_pool(name="stB", bufs=1, space="PSUM"))
        acc = ctx.enter_context(tc.tile_pool(name="acc", bufs=3, space="PSUM"))

        otoks = {}
        otns = {}
        obs = {}

        def qp_chunk(f, q):
            ps = acc.tile([128, 512], dt.float32, tag="acc", name=f"qps{f}_{q}")
            for k in range(KC):
                nc.tensor.matmul(
                    ps[:], lhsT=wqs[k][:, f * 128:(f + 1) * 128],
                    rhs=xts2[k][q // 2][:, (q % 2) * 512:(q % 2 + 1) * 512],
                    start=(k == 0), stop=(k == KC - 1))
            nc.vector.tensor_copy(QT[f][q][:], ps[:])

        def tr_chunk(isub):
            ib, col = isub // 4, (isub % 4) * 128
            trp = acc.tile([128, HPC * 65], dt.float32, tag="acc",
                           name=f"trp{isub}")
            for h in range(HPC):
                nc.tensor.transpose(trp[:, h * 65:(h + 1) * 65],
                                    OPS[h][ib][0:65, col:col + 128],
                                    id_f32[0:65, 0:65])
            trv = trp.rearrange("p (h c) -> p h c", c=65)
            linv6 = linvp.tile([128, HPC, 1], dt.float32, tag="l6",
                               name=f"l6{isub}")
            nc.vector.reciprocal(linv6[:], trv[:, :, 64:65])
            otok = otokp.tile([128, FPC], dt.bfloat16, tag="otok",
                              name=f"otok{isub}")
            a, b = broadcast_tensor_aps(trv[:, :, 0:64], linv6[:])
            nc.vector.tensor_mul(
                otok.rearrange("p (h c) -> p h c", c=64), a, b)
            otoks[isub] = otok

        def tb_chunk(isub):
            otok = otoks.pop(isub)
            tbp = acc.tile([128, FPC], dt.bfloat16, tag="acc",
                           name=f"tbp{isub}")
            for f in range(FT):
                nc.tensor.transpose(tbp[:, f * 128:(f + 1) * 128],
                                    otok[:, f * 128:(f + 1) * 128],
                                    id_bf[:])
            otn = otnp.tile([128, FPC], dt.bfloat16, tag="otn",
                            name=f"otn{isub}")
            nc.vector.tensor_copy(otn[:], tbp[:])
            otns[isub] = otn

        def pj_chunk(isub, half):
            otn = otns[isub]
            if half == 0:
                obs[isub] = outst.tile([128, DIM], dt.float32, tag="ob",
                                       name=f"ob{isub}")
            ob = obs[isub]
            pp = acc.tile([128, DIM // 2], dt.float32, tag="acc",
                          name=f"pp{isub}_{half}")
            for f in range(FT):
                nc.tensor.matmul(
                    pp[:], lhsT=otn[:, f * 128:(f + 1) * 128],
                    rhs=wos[f][:, half * 384:(half + 1) * 384],
                    start=(f == 0), stop=(f == FT - 1))
            nc.vector.tensor_copy(ob[:, half * 384:(half + 1) * 384], pp[:])
            if half == 1:
                otns.pop(isub)
                nc.sync.dma_start(out=out[isub * 128:(isub + 1) * 128, :],
                                  in_=obs.pop(isub)[:])

        def window_fillers(w):
            fills = []
            qps = [(f, w + 1) for f in range(FT)] if w + 1 < IB else []
            p3 = []
            if w >= 1:
                for i in range(4):
                    isub = 4 * (w - 1) + i
                    p3.append((tr_chunk, (isub,)))
                    p3.append((tb_chunk, (isub,)))
                    p3.append((pj_chunk, (isub, 0)))
                    p3.append((pj_chunk, (isub, 1)))
            # interleave qp early among p3 chunks
            for j, qpq in enumerate(qps):
                p3.insert(min(len(p3), 1 + 5 * j), (qp_chunk, qpq))
            return p3

        # units: ib-outer so OPS[:, ib] completes at end of window ib
        units = [(h, ib, jc) for ib in range(IB) for h in range(HPC)
                 for jc in range(NT)]
        chunks = []
        i = 0
        tog = True
        while i < len(units):
            n = min(3 if tog else 2, len(units) - i)
            chunks.append(units[i:i + n])
            i += n
            tog = not tog

        ops = {}
        exs = {}

        def emit_scores(g, pack):
            n = len(pack)
            pool = stA if n == 3 else stB
            st = pool.tile([128, n * IBS], dt.float32,
                           tag="stA" if n == 3 else "stB", name=f"st{g}")
            for u, (h, ib, jc) in enumerate(pack):
                r0 = (h % 2) * 64
                nc.tensor.matmul(
                    st[:, u * IBS:(u + 1) * IBS],
                    lhsT=KT[h // 2][r0:r0 + 64, jc * 128:(jc + 1) * 128],
                    rhs=QT[h // 2][ib][r0:r0 + 64, :],
                    start=True, stop=True)
            ex = expp.tile([128, n * IBS], dt.bfloat16, tag="ex",
                           name=f"ex{g}")
            nc.scalar.activation(ex[:], st[:], EXP, scale=0.125)
            exs[g] = ex

        def emit_av(g, pack):
            ex = exs.pop(g)
            for u, (h, ib, jc) in enumerate(pack):
                if jc == 0:
                    ops[(h, ib)] = acc.tile([128, IBS], dt.float32,
                                            tag="acc", name=f"opp{h}_{ib}")
                nc.tensor.matmul(
                    ops[(h, ib)][:], lhsT=VP[jc][:, h * 128:(h + 1) * 128],
                    rhs=ex[:, u * IBS:(u + 1) * IBS],
                    start=(jc == 0), stop=(jc == NT - 1))
                if jc == NT - 1:
                    op = ops.pop((h, ib))
                    nc.vector.tensor_copy(OPS[h][ib][:], op[0:65, :])

        # per-window filler slots, spread evenly across the window's chunks
        win_of_chunk = [pack[0][1] for pack in chunks]
        n_in_win = [win_of_chunk.count(w) for w in range(IB)]
        fillers = []
        fill_due = {}  # chunk index -> count of fillers to emit after it
        cidx0 = 0
        for w in range(IB):
            fl = window_fillers(w)
            nw = n_in_win[w]
            for j in range(len(fl)):
                # place filler j after local chunk floor((j+1)*nw/(nf+1))
                loc = min(nw - 1, (j + 1) * nw // (len(fl) + 1))
                fill_due[cidx0 + loc] = fill_due.get(cidx0 + loc, 0) + 1
            fillers.extend(fl)
            cidx0 += nw

        fq = list(fillers)
        for g, pack in enumerate(chunks):
            emit_scores(g, pack)
            for _ in range(fill_due.get(g, 0)):
                fn, args = fq.pop(0)
                fn(*args)
            if g > 0:
                emit_av(g - 1, chunks[g - 1])
        emit_av(len(chunks) - 1, chunks[-1])
        for fn, args in fq:
            fn(*args)
        # tail: P3 for the last window
        for i in range(4):
            isub = 4 * (IB - 1) + i
            tr_chunk(isub)
            tb_chunk(isub)
            pj_chunk(isub, 0)
            pj_chunk(isub, 1)

    nc.finalize()
    return nc


def _get_nc():
    if "nc" not in _cache:
        _cache["nc"] = _build_nc()
    return _cache["nc"]


def kernel(x, Wq, Wk, Wv, Wo, bo):
    global last_exec_time_ns
    x = np.asarray(x, dtype=np.float32)
    Wq = np.asarray(Wq, dtype=np.float32)
    Wk = np.asarray(Wk, dtype=np.float32)
    Wv = np.asarray(Wv, dtype=np.float32)
    Wo = np.asarray(Wo, dtype=np.float32)
    bo = np.asarray(bo, dtype=np.float32)

    trace = bool(os.environ.get("BASS_KERNEL_TRACE"))
    if trace:
        _install_ntff_hook()
        import concourse.bass_utils as bass_utils
        bass_utils.upload_artifacts = lambda tmpdir: tmpdir

    nc = _get_nc()
    in_maps = []
    for c in range(NCORES):
        bi, hg = divmod(c, 2)
        s = slice(hg * FPC, (hg + 1) * FPC)
        in_maps.append({
            "xT": np.ascontiguousarray(x[bi].T).astype(BF16),
            "wq": np.ascontiguousarray(Wq[:, s]).astype(BF16),
            "wk": np.ascontiguousarray(Wk[:, s]).astype(BF16),
            "wv": np.ascontiguousarray(Wv[:, s]).astype(BF16),
            "wo": np.ascontiguousarray(Wo[s, :]).astype(BF16),
        })

    from concourse.bass_utils import run_bass_kernel_spmd
    res = run_bass_kernel_spmd(nc, in_maps, list(range(NCORES)), trace=trace)
    last_exec_time_ns = res.exec_time_ns

    parts = [res.results[c]["out"] for c in range(NCORES)]
    full = np.empty((B, N, DIM), np.float32)
    for bi in range(B):
        full[bi] = parts[2 * bi] + parts[2 * bi + 1] + bo[None, :]
    return full
